# revision 5
# baseline (speedup 1.0000x reference)
"""Trainium2 Bass kernel for CenterWoParamMultiCosineNearLoss.

loss = mean_b [ S_b - m_b + (2*m_b^2 - Q_b) / S_b ]   where, per sample b,
  d_k = 1 - <x_b, c_{label_b, k}>  (k = 0..15 sub-centers of own class)
  S = sum_k d_k, Q = sum_k d_k^2, m = min_k d_k
(algebraically identical to the reference's term1+term2).

Sharding: samples are sorted by label on the host (the loss is a mean over
samples, hence permutation-invariant) and split into 8 contiguous shards of
1024 — data-parallel with class-clustered assignment. Each 128-row block of
a shard spans only `spanw` (2-4) consecutive classes, so every block's
matmul streams just its own small center window (spanw*16 columns, packed
per block on the host) instead of all 90*16 columns.

x and centers ship as fp8e4m3 scaled by 16. Row statistics come from one
fused tensor_tensor_reduce per quantity per block:
  ssum = sum_k s_k   via multiplicative one-hot mask {0,1}  (scale 1/256)
  qsum = sum_k s_k^2 via squaring the masked row
  mx   = max_k s_k   via additive mask {0,-448} (offset keeps other
                     columns below any real similarity after descale)
The masks are host-precomputed fp8 tensors DMA'd up front, so the vector
engine runs exactly 3 instructions per block and nothing depends on a slow
SWDGE label transfer. Each core DMAs back a [128, 24] stats tensor
(ssum/qsum/mx per row); the host applies the closed-form per-row loss and
means over the batch (the "all-reduce the scalar loss" step of the
sharding hint, in fp64).

Perf notes (from perfetto traces): dummy warm-up matmuls hold the PE HAM
clock-gate open through the DMA phase; DMA descriptor generation is split
across both HWDGE engines (Sync + Activation) so it is not serialized; no
scalar-engine activations -> no ACT_TABLE_LOAD; fp8 + FWL gives 30ns
back-to-back matmuls at N=48.
"""

import os
import sys

import numpy as np

for _p in ("/opt/trn_rl_repo", "/root/.axon_site/_ro/trn_rl_repo"):
    if os.path.isdir(_p) and _p not in sys.path:
        sys.path.append(_p)

import ml_dtypes  # noqa: E402

import concourse.tile as tile  # noqa: E402
from concourse import bacc  # noqa: E402
from concourse import mybir  # noqa: E402
from concourse.bass_utils import run_bass_kernel_spmd  # noqa: E402

P = 128          # SBUF partitions
B = 8192         # batch
D = 1024         # feature dim
C = 90           # classes
K = 16           # sub-centers per class
NCORES = 8
SHARD = B // NCORES          # 1024 samples per core
NB = SHARD // P              # 8 row-blocks per core
KT = D // P                  # 8 contraction tiles
NCH = 4                      # x DMA chunks (2 blocks each)
BPC = NB // NCH              # blocks per chunk
NWARM = 5                    # PE warm-up matmuls (N=512 each)

SCALE = 16.0                 # fp8 quantization scale for x and centers
DESCALE = 1.0 / (SCALE * SCALE)
MNEG = -240.0                # additive mask for the max-reduce (fp8e4m3 max)

_F32 = mybir.dt.float32
_F8 = mybir.dt.float8e4
_NP_F8 = ml_dtypes.float8_e4m3

_ADD = mybir.AluOpType.add
_MULT = mybir.AluOpType.mult
_MAX = mybir.AluOpType.max


def _build_program(spanw: int):
    """One SPMD program for all 8 cores. spanw = classes per block window."""
    spanc = spanw * K
    nc = bacc.Bacc(None, target_bir_lowering=False)
    xT = nc.declare_dram_parameter("xT", [NCH, P, KT, BPC * P], _F8, isOutput=False)
    cw = nc.declare_dram_parameter("cw", [P, KT, NB, spanc], _F8, isOutput=False)
    m01 = nc.declare_dram_parameter("m01", [P, NB, spanc], _F8, isOutput=False)
    mng = nc.declare_dram_parameter("mng", [P, NB, spanc], _F8, isOutput=False)
    out = nc.declare_dram_parameter("out", [P, 3 * NB], _F32, isOutput=True)

    with tile.TileContext(nc) as tc:
        with (
            tc.tile_pool(name="const", bufs=1) as const,
            tc.tile_pool(name="cwp", bufs=1) as cwp,
            tc.tile_pool(name="xp", bufs=NCH) as xp,
            tc.tile_pool(name="maskp", bufs=1) as maskp,
            tc.tile_pool(name="work", bufs=6) as work,
            tc.tile_pool(name="stats", bufs=1) as stats,
            tc.tile_pool(name="wpp", bufs=1, space="PSUM") as wpp,
            tc.tile_pool(name="pp", bufs=6, space="PSUM") as pp,
        ):
            # --- PE warm-up: hold HAM at K=8/8 through the DMA phase ---
            wsrc = const.tile([P, 512], _F8)
            nc.vector.memset(wsrc[:, :], 0.0)
            wps = wpp.tile([P, 512], _F32)
            for _ in range(NWARM):
                nc.tensor.matmul(
                    wps[:, :], lhsT=wsrc[:, 0:P], rhs=wsrc[:, :],
                    start=True, stop=True,
                )

            # --- bulk DMAs, desc-gen split across the two HWDGE engines ---
            # Act ring: cw is needed by the very first matmul; x chunks 0/2.
            # Sync ring: masks (gate the DVE pipeline), x chunks 1/3, out.
            cwt = cwp.tile([P, KT, NB, spanc], _F8)
            nc.scalar.dma_start(out=cwt[:, :, :, :], in_=cw[:, :, :, :])
            m01t = maskp.tile([P, NB, spanc], _F8)
            nc.sync.dma_start(out=m01t[:, :, :], in_=m01[:, :, :])
            mngt = maskp.tile([P, NB, spanc], _F8)
            nc.sync.dma_start(out=mngt[:, :, :], in_=mng[:, :, :])
            xtiles = []
            for j in range(NCH):
                xc = xp.tile([P, KT, BPC * P], _F8, tag="xc")
                eng = nc.scalar if j % 2 == 0 else nc.sync
                eng.dma_start(out=xc[:, :, :], in_=xT[j, :, :, :])
                xtiles.append(xc)

            # stats layout: cols [0,NB)=ssum, [NB,2NB)=qsum, [2NB,3NB)=mx
            st = stats.tile([P, 3 * NB], _F32)

            for j in range(NCH):
                for h in range(BPC):
                    b = BPC * j + h
                    ps = pp.tile([P, spanc], _F32)
                    for k in range(KT):
                        nc.tensor.matmul(
                            ps[:, :],
                            lhsT=xtiles[j][:, k, h * P : (h + 1) * P],
                            rhs=cwt[:, k, b, :],
                            start=(k == 0),
                            stop=(k == KT - 1),
                        )
                    # fused mask-select + row reduce, one DVE op per stat
                    sm = work.tile([P, spanc], _F32, tag="sm")
                    nc.vector.tensor_tensor_reduce(
                        out=sm[:, :], in0=ps[:, :], in1=m01t[:, b, :],
                        scale=DESCALE, scalar=0.0, op0=_MULT, op1=_ADD,
                        accum_out=st[:, b : b + 1],
                    )
                    sq = work.tile([P, spanc], _F32, tag="sq")
                    nc.vector.tensor_tensor_reduce(
                        out=sq[:, :], in0=sm[:, :], in1=sm[:, :],
                        scale=1.0, scalar=0.0, op0=_MULT, op1=_ADD,
                        accum_out=st[:, NB + b : NB + b + 1],
                    )
                    s2 = work.tile([P, spanc], _F32, tag="s2")
                    nc.vector.tensor_tensor_reduce(
                        out=s2[:, :], in0=ps[:, :], in1=mngt[:, b, :],
                        scale=DESCALE, scalar=-4.0, op0=_ADD, op1=_MAX,
                        accum_out=st[:, 2 * NB + b : 2 * NB + b + 1],
                    )

            nc.sync.dma_start(out=out[:, :], in_=st[:, :])

    nc.finalize()
    return nc


def _prep_inputs(x, labels, centers):
    """Host-side sharding/layout prep. Returns (in_maps, spanw)."""
    labels = np.asarray(labels).astype(np.int64)
    x = np.ascontiguousarray(np.asarray(x, dtype=np.float32))
    centers = np.asarray(centers, dtype=np.float32)

    perm = np.argsort(labels, kind="stable")
    ls = labels[perm]

    # per-(core, block) class windows
    los = np.empty((NCORES, NB), dtype=np.int64)
    spans = np.empty((NCORES, NB), dtype=np.int64)
    for i in range(NCORES):
        seg = ls[i * SHARD : (i + 1) * SHARD]
        for b in range(NB):
            blk = seg[b * P : (b + 1) * P]
            los[i, b] = blk[0]
            spans[i, b] = blk[-1] - blk[0] + 1
    spanw = int(spans.max())
    assert spanw * K <= 512, f"block class span {spanw} too large"
    los = np.minimum(los, C - spanw)

    centersT = centers.reshape(C * K, D).T            # [D, C*K]
    xq = (x * SCALE).astype(_NP_F8)                   # quantize once

    spanc = spanw * K
    in_maps = []
    for i in range(NCORES):
        rows = perm[i * SHARD : (i + 1) * SHARD]
        xsT = np.ascontiguousarray(xq[rows].T)        # [D, SHARD] fp8
        xdev = np.ascontiguousarray(
            xsT.reshape(KT, P, NCH, BPC * P).transpose(2, 1, 0, 3)
        )                                             # [NCH, P, KT, 256]
        # per-block center windows, k-major cols: col j = k*spanw + c
        cwdev = np.empty((P, KT, NB, spanc), dtype=_NP_F8)
        lab_core = ls[i * SHARD : (i + 1) * SHARD]
        m01dev = np.zeros((P, NB, spanc), dtype=_NP_F8)
        mngdev = np.full((P, NB, spanc), MNEG, dtype=_NP_F8)
        kcol = (np.arange(spanc) // spanw)            # unused; cols are k-major
        ccol = (np.arange(spanc) % spanw)             # class id of each column
        for b in range(NB):
            lo = int(los[i, b])
            win = centersT[:, K * lo : K * (lo + spanw)]      # [D, spanw*K] c-major
            winq = (win * SCALE).astype(_NP_F8)
            wkm = winq.reshape(D, spanw, K).transpose(0, 2, 1).reshape(D, spanc)
            cwdev[:, :, b, :] = wkm.reshape(KT, P, spanc).transpose(1, 0, 2)
            loc = (lab_core[b * P : (b + 1) * P] - lo)        # [P] in [0, spanw)
            hit = loc[:, None] == ccol[None, :]               # [P, spanc]
            m01dev[:, b, :] = hit.astype(_NP_F8)
            mngdev[:, b, :][hit] = 0.0
        in_maps.append({
            "xT": xdev,
            "cw": np.ascontiguousarray(cwdev),
            "m01": m01dev,
            "mng": mngdev,
        })
    return in_maps, spanw


def _finish(results):
    """Host epilogue: closed-form per-row loss from the [P, 3*NB] stats."""
    ssum = np.concatenate([r["out"][:, 0:NB].astype(np.float64).ravel()
                           for r in results])
    qsum = np.concatenate([r["out"][:, NB:2 * NB].astype(np.float64).ravel()
                           for r in results])
    mx = np.concatenate([r["out"][:, 2 * NB:3 * NB].astype(np.float64).ravel()
                         for r in results])
    S = K - ssum
    Q = K - 2.0 * ssum + qsum
    m = 1.0 - mx
    rl = S - m + (2.0 * m * m - Q) / S
    return np.float32(rl.mean())


def kernel(x, labels, centers):
    in_maps, spanw = _prep_inputs(x, labels, centers)
    nc = _build_program(spanw)
    res = run_bass_kernel_spmd(nc, in_maps, core_ids=list(range(NCORES)))
    return _finish(res.results)
